# revision 34
# baseline (speedup 1.0000x reference)
"""Trainium2 Bass kernel for nn_AttentionBlock (B=4, H=W=64, C=256, D=32).

Sharding: 8 shards = 4 samples x 2 query-halves. Host pre-transposes x to
channel-major (xT) per core, so the kernel starts straight into the
projections. Each core computes K/V for all 4096 keys and attention +
output projection for its 2048 queries. Residual and bias folds are done
on host (exact, f32). No collectives.

Device structure per core:
  - q/k/v projections from xT (kT packed into 2 row-strips for 2x
    row-tiled S matmuls; qT replicated on partitions 0-63).
  - 4 supergroups of 512 queries; per supergroup 16 "sets" of 2 key
    chunks: S = kT^T @ qT via two concurrent 32x128 row-tiled matmuls,
    exp on ACT (scaled by 2^-4, cancels in normalization), attend
    accumulated into per-128-query psum tiles with a ones-column for the
    softmax denominator, software-pipelined one set behind S/exp.
  - epilogue: normalize, transpose via PE, output projection, staged
    store (one DMA per supergroup).

PSUM: pool "ps" tag ps [128,1024] f32 x2 bufs = 4 banks (S sets + phase B),
pool "ps_att" tag a [128,512] f32 x4 bufs = 4 banks (pa accumulators +
epilogue transpose/out-proj scratch). Total 8 banks.

Self-contained: hardcodes shapes, imports only /opt/trn_rl_repo concourse.
"""

import sys

if "/opt/trn_rl_repo" not in sys.path:
    sys.path.insert(0, "/opt/trn_rl_repo")

import numpy as np
import ml_dtypes

BF16 = ml_dtypes.bfloat16

# Problem constants
B, HH, WW, C = 4, 64, 64, 256
D = 32
N = HH * WW           # 4096 keys per sample
NQ = N // 2           # 2048 queries per core
NCORES = 8
KC = N // 128         # 32 key chunks
NSETS = KC // 2       # 16 sets of 2 chunks
NSG = NQ // 512       # 4 supergroups of 512 queries
EXP_BIAS = float(-4.0 * np.log(2.0))  # et = exp(s) * 2^-4 (cancels in softmax)
FEXP_A = float(2**23 / np.log(2.0))  # Schraudolph fast-exp (DVE offload path)
FEXP_B = 1031438784.0                # (127-4)*2^23 - 360000, RNE-tuned
FEXP_EVERY = 3                       # every 3rd slot's exp runs on DVE+gpsimd
USE_FP8 = False  # fp8e4 et/vsb + DoubleRow attend (slower: DR<->tile mode switches)
PACK = 2  # S row-tiling: 4 -> 256q groups/4-chunk sets; 2 -> 512q/2-chunk sets

_compiled_cache = {}


def _build(use_bias: bool):
    from contextlib import ExitStack
    from concourse import bacc, tile, mybir, masks

    f32 = mybir.dt.float32
    bf = mybir.dt.bfloat16
    fp8 = mybir.dt.float8e4
    edt = fp8 if USE_FP8 else bf
    VSW = 272 if USE_FP8 else 260  # vsb row stride (DR rhs needs %16 bytes)

    QG = 1024 // PACK        # queries per group (set = PACK chunks x QG = 1024 psum cols)
    NG = NQ // QG            # groups
    SETS = KC // PACK        # sets per group
    NQH = QG // 128          # 128-query blocks per group
    total = NG * SETS        # 64 slots either way

    nc = bacc.Bacc("TRN2", target_bir_lowering=False, debug=False, num_devices=NCORES)

    VW = 257 if use_bias else 256  # v-proj output cols (col 256 = r for bias fold)

    xT_d = nc.dram_tensor("xT16", [2, 128, N], bf, kind="ExternalInput")
    wqp_d = nc.dram_tensor("wqp", [2, 128, 32 * PACK], bf, kind="ExternalInput")
    wkp_d = nc.dram_tensor("wkp", [2, 128, 32 * PACK], bf, kind="ExternalInput")
    wvp_d = nc.dram_tensor("wvp", [2, 128, VW], bf, kind="ExternalInput")
    wop_d = nc.dram_tensor("wop", [2, 128, 256], bf, kind="ExternalInput")
    out_d = nc.dram_tensor("out", [NQ, C], f32, kind="ExternalOutput")

    Exp = mybir.ActivationFunctionType.Exp
    Mult = mybir.AluOpType.mult
    Add = mybir.AluOpType.add
    i32 = mybir.dt.int32

    with tile.TileContext(nc) as tc:
        with ExitStack() as ctx:
            const = ctx.enter_context(tc.tile_pool(name="const", bufs=1))
            expp = ctx.enter_context(tc.tile_pool(name="expp", bufs=4))
            i32p = ctx.enter_context(tc.tile_pool(name="i32p", bufs=3))
            small = ctx.enter_context(tc.tile_pool(name="small", bufs=3))
            ps = ctx.enter_context(tc.tile_pool(name="ps", bufs=2, space="PSUM"))
            ps_att = ctx.enter_context(tc.tile_pool(name="ps_att", bufs=4, space="PSUM"))

            # ---- constants & weights ----
            ident = const.tile([128, 128], bf, tag="ident")
            masks.make_identity(nc, ident[:])
            ebias = const.tile([128, 1], f32, tag="ebias")
            nc.gpsimd.memset(ebias[:], EXP_BIAS)

            WQ = 32 * PACK
            wq0 = const.tile([128, WQ], bf, tag="wq0")
            wq1 = const.tile([128, WQ], bf, tag="wq1")
            wk0 = const.tile([128, WQ], bf, tag="wk0")
            wk1 = const.tile([128, WQ], bf, tag="wk1")
            wv0 = const.tile([128, VW], bf, tag="wv0")
            wv1 = const.tile([128, VW], bf, tag="wv1")
            wo0 = const.tile([128, 256], bf, tag="wo0")
            wo1 = const.tile([128, 256], bf, tag="wo1")
            # weights + xT on the hardware DMA queues (sync/scalar), in
            # need-order: k -> first tokens -> q -> v -> remaining tokens -> o.
            # (gpsimd dma_start is a slow SWDGE path - weights arrived ~150us
            # late there and stalled the epilogues.)
            xT = const.tile([128, 2, N], bf, tag="xT")
            wdma = [
                (wk0, wkp_d, 0), (wq0, wqp_d, 0), (wv0, wvp_d, 0), (wo0, wop_d, 0),
                (wk1, wkp_d, 1), (wq1, wqp_d, 1), (wv1, wvp_d, 1), (wo1, wop_d, 1),
            ]
            for j in range(4):
                sl = slice(1024 * j, 1024 * j + 1024)
                w, dram, hi = wdma[j]
                nc.sync.dma_start(out=w[:], in_=dram[hi, :, :])
                nc.sync.dma_start(out=xT[:, 0, sl], in_=xT_d[0, :, sl])
                w, dram, hi = wdma[4 + j]
                nc.scalar.dma_start(out=w[:], in_=dram[hi, :, :])
                nc.scalar.dma_start(out=xT[:, 1, sl], in_=xT_d[1, :, sl])

            # SBUF destinations
            qT = const.tile([32 * PACK, NQ], bf, tag="qT")    # PACK replicas of q^T
            kT = const.tile([32 * PACK, SETS * 128], bf, tag="kT")  # strip j = chunks PACK*u+j
            vsb = const.tile([128, KC, VSW], edt, tag="vsb")
            stage = const.tile([128, 16, 256], f32, tag="stage")
            nc.gpsimd.memset(vsb[:, :, 256:257], 1.0)

            def q_proj(j):  # 512-token chunk j of the 2048 queries
                pq = ps.tile([128, 1024], f32, tag="ps", name=f"pq{j}")
                sl = slice(512 * j, 512 * j + 512)
                nc.tensor.matmul(pq[0:WQ, 0:512], wq0[:], xT[:, 0, sl], start=True, stop=False)
                nc.tensor.matmul(pq[0:WQ, 0:512], wq1[:], xT[:, 1, sl], start=False, stop=True)
                nc.vector.tensor_copy(qT[:, sl], pq[0:WQ, 0:512])

            def k_proj(s):  # 512-token chunk s of all 4096 keys
                pk = ps.tile([128, 1024], f32, tag="ps", name=f"pk{s}")
                sl = slice(512 * s, 512 * s + 512)
                nc.tensor.matmul(pk[0:WQ, 0:512], wk0[:], xT[:, 0, sl], start=True, stop=False)
                nc.tensor.matmul(pk[0:WQ, 0:512], wk1[:], xT[:, 1, sl], start=False, stop=True)
                # strip-pack: chunk PACK*u+j -> partitions 32j; tile has 4 chunks
                npc = 4 // PACK  # positions per strip within this 512-token tile
                for j in range(PACK):
                    for a in range(npc):
                        ch = npc * j + a if PACK == 4 else 2 * a + j  # within-tile chunk
                        u = (4 * s + ch) // PACK
                        seg = pk[32 * j : 32 * j + 32, 128 * ch : 128 * ch + 128]
                        dst = kT[32 * j : 32 * j + 32, 128 * u : 128 * u + 128]
                        if j % 2 == 0:
                            nc.vector.tensor_copy(dst, seg)
                        else:
                            nc.scalar.copy(dst, seg)

            def v_proj(m2, dve):  # pair of 128-token chunks; evac engine selectable
                pv = ps.tile([128, 1024], f32, tag="ps", name=f"pv{m2}")
                for r in range(2):
                    m = 2 * m2 + r
                    osl = pv[:, 512 * r : 512 * r + VW]
                    tsl = slice(128 * m, 128 * m + 128)
                    nc.tensor.matmul(osl, xT[:, 0, tsl], wv0[:], start=True, stop=False)
                    nc.tensor.matmul(osl, xT[:, 1, tsl], wv1[:], start=False, stop=True)
                for r in range(2):
                    m = 2 * m2 + r
                    osl = pv[:, 512 * r : 512 * r + VW]
                    if use_bias:
                        rv = small.tile([128, 1], f32, tag="rv")
                        nc.scalar.activation(rv[:], osl[:, 256:257], Exp)
                        nc.vector.tensor_scalar(vsb[:, m, 0:256], osl[:, 0:256], rv[:], None, Mult)
                        nc.vector.tensor_copy(vsb[:, m, 256:257], rv[:])
                    elif dve:
                        nc.vector.tensor_copy(vsb[:, m, 0:256], osl[:, 0:256])
                    else:
                        nc.scalar.copy(vsb[:, m, 0:256], osl[:, 0:256])

            def epilogue_batch(g, pas, ats):
                # normalize (rec/at) was emitted with the last attend set;
                # the freed pa tiles double as transpose/out-proj psum scratch.
                for qh in range(NQH):
                    pa, at = pas[qh], ats[qh]
                    nc.tensor.matmul(pa[:, 256:384], at[:, 0:128], ident[:], start=True, stop=True)
                    nc.tensor.matmul(pa[:, 384:512], at[:, 128:256], ident[:], start=True, stop=True)
                aTs = []
                for qh in range(NQH):
                    aT = small.tile([128, 256], bf, tag="aT", bufs=4)
                    nc.vector.tensor_copy(aT[:], pas[qh][:, 256:512])
                    aTs.append(aT)
                for qh in range(NQH):
                    pa, aT = pas[qh], aTs[qh]
                    nc.tensor.matmul(pa[:, 0:256], aT[:, 0:128], wo0[:, 0:256], start=True, stop=False)
                    nc.tensor.matmul(pa[:, 0:256], aT[:, 128:256], wo1[:, 0:256], start=False, stop=True)
                for qh in range(NQH):
                    nc.vector.tensor_copy(stage[:, NQH * g + qh, :], pas[qh][:, 0:256])
                nc.sync.dma_start(
                    out=out_d[:].rearrange("(t p) c -> p t c", p=128)[:, NQH * g : NQH * g + NQH, :],
                    in_=stage[:, NQH * g : NQH * g + NQH, :],
                )

            # ---- head: all projections (ACT idle; share evac DVE/ACT) ----
            k_proj(0)
            k_proj(1)
            q_proj(0)
            for s in range(2, 8):
                k_proj(s)
            v_proj(0, dve=True)
            v_proj(1, dve=False)

            # ---- phase C: S -> exp -> attend pipelined 2 sets deep ----
            pa_tiles = {}
            at_tiles = {}
            ets = {}
            epi_pending = None

            for si in range(total + 2):
                pst = None
                if si < total:
                    g, t = divmod(si, SETS)
                    qsl = slice(QG * g, QG * g + QG)
                    pst = ps.tile([128, 1024], f32, tag="ps", name=f"pst{si}")
                    for j in range(PACK):
                        nc.tensor.matmul(
                            pst[:, QG * j : QG * j + QG],
                            kT[32 * j : 32 * j + 32, 128 * t : 128 * t + 128],
                            qT[32 * j : 32 * j + 32, qsl],
                            start=True, stop=True, tile_position=(32 * j, 0),
                        )
                    if si < 14:
                        v_proj(si + 2, dve=(si % 3 != 2))
                    if si in (14, 28, 44):
                        q_proj(si // 16 + 1)
                # attend two sets behind S/exp
                if si - 2 in ets:
                    et_p, g_p, t_p = ets.pop(si - 2)
                    if t_p == 0:
                        for qh in range(NQH):
                            pa_tiles[(g_p, qh)] = ps_att.tile(
                                [128, 512], f32, tag="a", name=f"pa{g_p}_{qh}"
                            )
                    for qh in range(NQH):
                        pa = pa_tiles[(g_p, qh)]
                        if USE_FP8:
                            etp = et_p.rearrange("p (r c q) -> p r c q", r=PACK // 2, c=2)
                            for r in range(PACK // 2):
                                nc.tensor.matmul(
                                    pa[:, 0:257],
                                    etp[:, r, :, 128 * qh : 128 * qh + 128],
                                    vsb[:, PACK * t_p + 2 * r : PACK * t_p + 2 * r + 2, 0:257],
                                    start=(t_p == 0 and r == 0),
                                    stop=(t_p == SETS - 1 and r == PACK // 2 - 1),
                                    perf_mode=mybir.MatmulPerfMode.DoubleRow,
                                )
                        else:
                            for j in range(PACK):
                                nc.tensor.matmul(
                                    pa[:, 0:257],
                                    et_p[:, QG * j + 128 * qh : QG * j + 128 * qh + 128],
                                    vsb[:, PACK * t_p + j, 0:257],
                                    start=(t_p == 0 and j == 0),
                                    stop=(t_p == SETS - 1 and j == PACK - 1),
                                )
                        if t_p == SETS - 1:
                            rec = small.tile([128, 1], f32, tag="rec", bufs=4)
                            nc.vector.reciprocal(rec[:], pa[:, 256:257])
                            at = small.tile([128, 256], bf, tag="at", bufs=4)
                            nc.vector.tensor_scalar(at[:], pa[:, 0:256], rec[:], None, Mult)
                            at_tiles[(g_p, qh)] = at
                    if t_p == SETS - 1:
                        epi_pending = g_p
                if si < total:
                    et = expp.tile([128, 1024], edt, tag="e")
                    if si % FEXP_EVERY == FEXP_EVERY - 1:
                        it = i32p.tile([128, 1024], i32, tag="i")
                        nc.vector.tensor_scalar(it[:], pst[:], FEXP_A, FEXP_B, Mult, Add)
                        nc.gpsimd.tensor_copy(et[:], it[:].bitcast(f32))
                    else:
                        nc.scalar.activation(et[:], pst[:], Exp, bias=ebias[:])
                    ets[si] = (et, g, t)
                # epilogue after this slot's exp
                if epi_pending is not None:
                    epilogue_batch(
                        epi_pending,
                        [pa_tiles.pop((epi_pending, qh)) for qh in range(NQH)],
                        [at_tiles.pop((epi_pending, qh)) for qh in range(NQH)],
                    )
                    epi_pending = None

    nc.compile()
    return nc


def _get_compiled(use_bias: bool):
    key = bool(use_bias)
    if key not in _compiled_cache:
        _compiled_cache[key] = _build(use_bias)
    return _compiled_cache[key]


def _prep(x, wq, bq, wk, bk, wv, bv, wo, bo):
    xf = np.ascontiguousarray(np.asarray(x, dtype=np.float32)).reshape(B, N, C)
    wq = np.asarray(wq, np.float32)
    bq = np.asarray(bq, np.float32)
    wk = np.asarray(wk, np.float32)
    bk = np.asarray(bk, np.float32)
    wv = np.asarray(wv, np.float32)
    bv = np.asarray(bv, np.float32)
    wo = np.asarray(wo, np.float32)
    bo = np.asarray(bo, np.float32)

    use_bias = not (np.all(bq == 0) and np.all(bk == 0) and np.all(bv == 0))

    scale = np.float32(1.0 / np.sqrt(np.float32(D)))
    wqs = wq * scale
    # lhsT tiles: [2 c-halves, 128, 64] with d replicated 2x along columns
    wqp = np.ascontiguousarray(np.tile(wqs.reshape(2, 128, D), (1, 1, PACK))).astype(BF16)
    wkp = np.ascontiguousarray(np.tile(wk.reshape(2, 128, D), (1, 1, PACK))).astype(BF16)
    if use_bias:
        # scores row-fold: r_k = x_k @ u, u = scale * (wk @ bq); exp(r) scales
        # key k's et column (bk and bq*bk terms drop out of softmax).
        u = (wk @ (bq * scale)).astype(np.float32)  # [C]
        wvx = np.concatenate([wv, u[:, None]], axis=1)  # [C, 257]
        wvp = np.ascontiguousarray(wvx.reshape(2, 128, 257)).astype(BF16)
    else:
        wvp = np.ascontiguousarray(wv.reshape(2, 128, 256)).astype(BF16)
    wop = np.ascontiguousarray(wo.reshape(2, 128, 256)).astype(BF16)

    in_maps = []
    for core in range(NCORES):
        b, h = divmod(core, 2)
        if h == 0:
            xo = xf[b]
        else:
            xo = np.concatenate([xf[b, NQ:], xf[b, :NQ]], 0)
        xT = np.ascontiguousarray(xo.T.reshape(2, 128, N)).astype(BF16)
        in_maps.append(
            {"xT16": xT, "wqp": wqp, "wkp": wkp, "wvp": wvp, "wop": wop}
        )
    # host residual fold: out += x + (bv @ wo + bo)
    resid_const = (bv.astype(np.float64) @ wo.astype(np.float64)).astype(np.float32) + bo
    return in_maps, use_bias, xf, resid_const


def _gather(results, xf, resid_const):
    out = np.empty((B, N, C), np.float32)
    for core in range(NCORES):
        b, h = divmod(core, 2)
        out[b, NQ * h : NQ * (h + 1)] = results[core]["out"]
    out += xf
    out += resid_const[None, None, :]
    return out.reshape(B, HH, WW, C)


def kernel(x, wq, bq, wk, bk, wv, bv, wo, bo):
    from concourse.bass_utils import run_bass_kernel_spmd

    in_maps, use_bias, xf, resid_const = _prep(x, wq, bq, wk, bk, wv, bv, wo, bo)
    nc = _get_compiled(use_bias)
    res = run_bass_kernel_spmd(nc, in_maps, core_ids=list(range(NCORES)))
    return _gather(res.results, xf, resid_const)


def _ensure_ntff_hook():
    """The agent image's antenv stub lacks axon_hooks; synthesize it so
    run_bass_kernel_spmd(trace=True) can NTFF-profile via libaxon_pjrt."""
    import types

    try:
        from antenv.axon_hooks import get_axon_ntff_profile_hook  # noqa: F401
        return
    except ImportError:
        pass
    import antenv
    from trn_agent_boot.trn_boot import _ntff_profile_via_ctypes

    mod = types.ModuleType("antenv.axon_hooks")
    state = {"h": _ntff_profile_via_ctypes("/opt/axon/libaxon_pjrt.so")}
    mod.get_axon_ntff_profile_hook = lambda: state["h"]
    mod.set_axon_ntff_profile_hook = lambda h: state.__setitem__("h", h)
    sys.modules["antenv.axon_hooks"] = mod
    antenv.axon_hooks = mod


def run_traced(inputs, **kw):
    """For test.py: run with NTFF profiling; returns (output, BassKernelResults)."""
    from concourse.bass_utils import run_bass_kernel_spmd

    _ensure_ntff_hook()

    in_maps, use_bias, xf, resid_const = _prep(**inputs)
    nc = _get_compiled(use_bias)
    res = run_bass_kernel_spmd(nc, in_maps, core_ids=list(range(NCORES)), trace=True, **kw)
    return _gather(res.results, xf, resid_const), res


# revision 35
# speedup vs baseline: 1.4126x; 1.4126x over previous
"""Trainium2 Bass kernel for nn_AttentionBlock (B=4, H=W=64, C=256, D=32).

Sharding: 8 shards = 4 samples x 2 query-halves. Host pre-transposes x to
channel-major (xT) per core, so the kernel starts straight into the
projections. Each core computes K/V for all 4096 keys and attention +
output projection for its 2048 queries. Residual and bias folds are done
on host (exact, f32). No collectives.

Device structure per core:
  - q/k/v projections from xT (kT packed into 2 row-strips for 2x
    row-tiled S matmuls; qT replicated on partitions 0-63).
  - 4 supergroups of 512 queries; per supergroup 16 "sets" of 2 key
    chunks: S = kT^T @ qT via two concurrent 32x128 row-tiled matmuls,
    exp on ACT (scaled by 2^-4, cancels in normalization), attend
    accumulated into per-128-query psum tiles with a ones-column for the
    softmax denominator, software-pipelined one set behind S/exp.
  - epilogue: normalize, transpose via PE, output projection, staged
    store (one DMA per supergroup).

PSUM: pool "ps" tag ps [128,1024] f32 x2 bufs = 4 banks (S sets + phase B),
pool "ps_att" tag a [128,512] f32 x4 bufs = 4 banks (pa accumulators +
epilogue transpose/out-proj scratch). Total 8 banks.

Self-contained: hardcodes shapes, imports only /opt/trn_rl_repo concourse.
"""

import sys

if "/opt/trn_rl_repo" not in sys.path:
    sys.path.insert(0, "/opt/trn_rl_repo")

import numpy as np
import ml_dtypes

BF16 = ml_dtypes.bfloat16

# Problem constants
B, HH, WW, C = 4, 64, 64, 256
D = 32
N = HH * WW           # 4096 keys per sample
NQ = N // 2           # 2048 queries per core
NCORES = 8
KC = N // 128         # 32 key chunks
NSETS = KC // 2       # 16 sets of 2 chunks
NSG = NQ // 512       # 4 supergroups of 512 queries
EXP_BIAS = float(-4.0 * np.log(2.0))  # et = exp(s) * 2^-4 (cancels in softmax)
FEXP_A = float(2**23 / np.log(2.0))  # Schraudolph fast-exp (DVE offload path)
FEXP_B = 1031438784.0                # (127-4)*2^23 - 360000, RNE-tuned
FEXP_EVERY = 4                       # every 4th slot's exp runs on DVE
USE_FP8 = False  # fp8e4 et/vsb + DoubleRow attend (slower: DR<->tile mode switches)
PACK = 2  # S row-tiling: 4 -> 256q groups/4-chunk sets; 2 -> 512q/2-chunk sets

_compiled_cache = {}


def _build(use_bias: bool):
    from contextlib import ExitStack
    from concourse import bacc, tile, mybir, masks

    f32 = mybir.dt.float32
    bf = mybir.dt.bfloat16
    fp8 = mybir.dt.float8e4
    edt = fp8 if USE_FP8 else bf
    VSW = 272 if USE_FP8 else 260  # vsb row stride (DR rhs needs %16 bytes)

    QG = 1024 // PACK        # queries per group (set = PACK chunks x QG = 1024 psum cols)
    NG = NQ // QG            # groups
    SETS = KC // PACK        # sets per group
    NQH = QG // 128          # 128-query blocks per group
    total = NG * SETS        # 64 slots either way

    nc = bacc.Bacc("TRN2", target_bir_lowering=False, debug=False, num_devices=NCORES)

    VW = 257 if use_bias else 256  # v-proj output cols (col 256 = r for bias fold)

    xT_d = nc.dram_tensor("xT16", [2, 128, N], bf, kind="ExternalInput")
    wqp_d = nc.dram_tensor("wqp", [2, 128, 32 * PACK], bf, kind="ExternalInput")
    wkp_d = nc.dram_tensor("wkp", [2, 128, 32 * PACK], bf, kind="ExternalInput")
    wvp_d = nc.dram_tensor("wvp", [2, 128, VW], bf, kind="ExternalInput")
    wop_d = nc.dram_tensor("wop", [2, 128, 256], bf, kind="ExternalInput")
    out_d = nc.dram_tensor("out", [NQ, C], f32, kind="ExternalOutput")

    Exp = mybir.ActivationFunctionType.Exp
    Mult = mybir.AluOpType.mult
    Add = mybir.AluOpType.add
    i32 = mybir.dt.int32

    with tile.TileContext(nc) as tc:
        with ExitStack() as ctx:
            const = ctx.enter_context(tc.tile_pool(name="const", bufs=1))
            expp = ctx.enter_context(tc.tile_pool(name="expp", bufs=4))
            i32p = ctx.enter_context(tc.tile_pool(name="i32p", bufs=3))
            small = ctx.enter_context(tc.tile_pool(name="small", bufs=3))
            ps = ctx.enter_context(tc.tile_pool(name="ps", bufs=2, space="PSUM"))
            ps_att = ctx.enter_context(tc.tile_pool(name="ps_att", bufs=4, space="PSUM"))

            # ---- constants & weights ----
            ident = const.tile([128, 128], bf, tag="ident")
            masks.make_identity(nc, ident[:])
            ebias = const.tile([128, 1], f32, tag="ebias")
            nc.gpsimd.memset(ebias[:], EXP_BIAS)

            WQ = 32 * PACK
            wq0 = const.tile([128, WQ], bf, tag="wq0")
            wq1 = const.tile([128, WQ], bf, tag="wq1")
            wk0 = const.tile([128, WQ], bf, tag="wk0")
            wk1 = const.tile([128, WQ], bf, tag="wk1")
            wv0 = const.tile([128, VW], bf, tag="wv0")
            wv1 = const.tile([128, VW], bf, tag="wv1")
            wo0 = const.tile([128, 256], bf, tag="wo0")
            wo1 = const.tile([128, 256], bf, tag="wo1")
            # weights + xT on the hardware DMA queues (sync/scalar), in
            # need-order: k -> first tokens -> q -> v -> remaining tokens -> o.
            # (gpsimd dma_start is a slow SWDGE path - weights arrived ~150us
            # late there and stalled the epilogues.)
            xT = const.tile([128, 2, N], bf, tag="xT")
            wdma = [
                (wk0, wkp_d, 0), (wq0, wqp_d, 0), (wv0, wvp_d, 0), (wo0, wop_d, 0),
                (wk1, wkp_d, 1), (wq1, wqp_d, 1), (wv1, wvp_d, 1), (wo1, wop_d, 1),
            ]
            for j in range(4):
                sl = slice(1024 * j, 1024 * j + 1024)
                w, dram, hi = wdma[j]
                nc.sync.dma_start(out=w[:], in_=dram[hi, :, :])
                nc.sync.dma_start(out=xT[:, 0, sl], in_=xT_d[0, :, sl])
                w, dram, hi = wdma[4 + j]
                nc.scalar.dma_start(out=w[:], in_=dram[hi, :, :])
                nc.scalar.dma_start(out=xT[:, 1, sl], in_=xT_d[1, :, sl])

            # SBUF destinations
            qT = const.tile([32 * PACK, NQ], bf, tag="qT")    # PACK replicas of q^T
            kT = const.tile([32 * PACK, SETS * 128], bf, tag="kT")  # strip j = chunks PACK*u+j
            vsb = const.tile([128, KC, VSW], edt, tag="vsb")
            stage = const.tile([128, 16, 256], f32, tag="stage")
            nc.gpsimd.memset(vsb[:, :, 256:257], 1.0)

            def q_proj(j):  # 512-token chunk j of the 2048 queries
                pq = ps.tile([128, 1024], f32, tag="ps", name=f"pq{j}")
                sl = slice(512 * j, 512 * j + 512)
                nc.tensor.matmul(pq[0:WQ, 0:512], wq0[:], xT[:, 0, sl], start=True, stop=False)
                nc.tensor.matmul(pq[0:WQ, 0:512], wq1[:], xT[:, 1, sl], start=False, stop=True)
                nc.vector.tensor_copy(qT[:, sl], pq[0:WQ, 0:512])

            def k_proj(s):  # 512-token chunk s of all 4096 keys
                pk = ps.tile([128, 1024], f32, tag="ps", name=f"pk{s}")
                sl = slice(512 * s, 512 * s + 512)
                nc.tensor.matmul(pk[0:WQ, 0:512], wk0[:], xT[:, 0, sl], start=True, stop=False)
                nc.tensor.matmul(pk[0:WQ, 0:512], wk1[:], xT[:, 1, sl], start=False, stop=True)
                # strip-pack: chunk PACK*u+j -> partitions 32j; tile has 4 chunks
                npc = 4 // PACK  # positions per strip within this 512-token tile
                for j in range(PACK):
                    for a in range(npc):
                        ch = npc * j + a if PACK == 4 else 2 * a + j  # within-tile chunk
                        u = (4 * s + ch) // PACK
                        seg = pk[32 * j : 32 * j + 32, 128 * ch : 128 * ch + 128]
                        dst = kT[32 * j : 32 * j + 32, 128 * u : 128 * u + 128]
                        if j % 2 == 0:
                            nc.vector.tensor_copy(dst, seg)
                        else:
                            nc.scalar.copy(dst, seg)

            def v_proj(m2, dve):  # pair of 128-token chunks; evac engine selectable
                pv = ps.tile([128, 1024], f32, tag="ps", name=f"pv{m2}")
                for r in range(2):
                    m = 2 * m2 + r
                    osl = pv[:, 512 * r : 512 * r + VW]
                    tsl = slice(128 * m, 128 * m + 128)
                    nc.tensor.matmul(osl, xT[:, 0, tsl], wv0[:], start=True, stop=False)
                    nc.tensor.matmul(osl, xT[:, 1, tsl], wv1[:], start=False, stop=True)
                for r in range(2):
                    m = 2 * m2 + r
                    osl = pv[:, 512 * r : 512 * r + VW]
                    if use_bias:
                        rv = small.tile([128, 1], f32, tag="rv")
                        nc.scalar.activation(rv[:], osl[:, 256:257], Exp)
                        nc.vector.tensor_scalar(vsb[:, m, 0:256], osl[:, 0:256], rv[:], None, Mult)
                        nc.vector.tensor_copy(vsb[:, m, 256:257], rv[:])
                    elif dve:
                        nc.vector.tensor_copy(vsb[:, m, 0:256], osl[:, 0:256])
                    else:
                        nc.scalar.copy(vsb[:, m, 0:256], osl[:, 0:256])

            def epilogue_batch(g, pas, ats):
                # normalize (rec/at) was emitted with the last attend set;
                # the freed pa tiles double as transpose/out-proj psum scratch.
                for qh in range(NQH):
                    pa, at = pas[qh], ats[qh]
                    nc.tensor.matmul(pa[:, 256:384], at[:, 0:128], ident[:], start=True, stop=True)
                    nc.tensor.matmul(pa[:, 384:512], at[:, 128:256], ident[:], start=True, stop=True)
                aTs = []
                for qh in range(NQH):
                    aT = small.tile([128, 256], bf, tag="aT", bufs=4)
                    nc.vector.tensor_copy(aT[:], pas[qh][:, 256:512])
                    aTs.append(aT)
                for qh in range(NQH):
                    pa, aT = pas[qh], aTs[qh]
                    nc.tensor.matmul(pa[:, 0:256], aT[:, 0:128], wo0[:, 0:256], start=True, stop=False)
                    nc.tensor.matmul(pa[:, 0:256], aT[:, 128:256], wo1[:, 0:256], start=False, stop=True)
                for qh in range(NQH):
                    nc.vector.tensor_copy(stage[:, NQH * g + qh, :], pas[qh][:, 0:256])
                nc.sync.dma_start(
                    out=out_d[:].rearrange("(t p) c -> p t c", p=128)[:, NQH * g : NQH * g + NQH, :],
                    in_=stage[:, NQH * g : NQH * g + NQH, :],
                )

            # ---- head: all projections (ACT idle; share evac DVE/ACT) ----
            k_proj(0)
            k_proj(1)
            q_proj(0)
            for s in range(2, 8):
                k_proj(s)
            v_proj(0, dve=True)
            v_proj(1, dve=False)

            # ---- phase C: S -> exp -> attend pipelined 2 sets deep ----
            pa_tiles = {}
            at_tiles = {}
            ets = {}
            epi_pending = None

            for si in range(total + 2):
                pst = None
                if si < total:
                    g, t = divmod(si, SETS)
                    qsl = slice(QG * g, QG * g + QG)
                    pst = ps.tile([128, 1024], f32, tag="ps", name=f"pst{si}")
                    for j in range(PACK):
                        nc.tensor.matmul(
                            pst[:, QG * j : QG * j + QG],
                            kT[32 * j : 32 * j + 32, 128 * t : 128 * t + 128],
                            qT[32 * j : 32 * j + 32, qsl],
                            start=True, stop=True, tile_position=(32 * j, 0),
                        )
                    if si < 14:
                        v_proj(si + 2, dve=(si % 3 != 2))
                    if si in (14, 28, 44):
                        q_proj(si // 16 + 1)
                # attend two sets behind S/exp
                if si - 2 in ets:
                    et_p, g_p, t_p = ets.pop(si - 2)
                    if t_p == 0:
                        for qh in range(NQH):
                            pa_tiles[(g_p, qh)] = ps_att.tile(
                                [128, 512], f32, tag="a", name=f"pa{g_p}_{qh}"
                            )
                    for qh in range(NQH):
                        pa = pa_tiles[(g_p, qh)]
                        if USE_FP8:
                            etp = et_p.rearrange("p (r c q) -> p r c q", r=PACK // 2, c=2)
                            for r in range(PACK // 2):
                                nc.tensor.matmul(
                                    pa[:, 0:257],
                                    etp[:, r, :, 128 * qh : 128 * qh + 128],
                                    vsb[:, PACK * t_p + 2 * r : PACK * t_p + 2 * r + 2, 0:257],
                                    start=(t_p == 0 and r == 0),
                                    stop=(t_p == SETS - 1 and r == PACK // 2 - 1),
                                    perf_mode=mybir.MatmulPerfMode.DoubleRow,
                                )
                        else:
                            for j in range(PACK):
                                nc.tensor.matmul(
                                    pa[:, 0:257],
                                    et_p[:, QG * j + 128 * qh : QG * j + 128 * qh + 128],
                                    vsb[:, PACK * t_p + j, 0:257],
                                    start=(t_p == 0 and j == 0),
                                    stop=(t_p == SETS - 1 and j == PACK - 1),
                                )
                        if t_p == SETS - 1:
                            rec = small.tile([128, 1], f32, tag="rec", bufs=4)
                            nc.vector.reciprocal(rec[:], pa[:, 256:257])
                            at = small.tile([128, 256], bf, tag="at", bufs=4)
                            nc.vector.tensor_scalar(at[:], pa[:, 0:256], rec[:], None, Mult)
                            at_tiles[(g_p, qh)] = at
                    if t_p == SETS - 1:
                        epi_pending = g_p
                if si < total:
                    et = expp.tile([128, 1024], edt, tag="e")
                    if si % FEXP_EVERY == FEXP_EVERY - 1:
                        it = i32p.tile([128, 1024], i32, tag="i")
                        nc.vector.tensor_scalar(it[:], pst[:], FEXP_A, FEXP_B, Mult, Add)
                        nc.vector.tensor_copy(et[:], it[:].bitcast(f32))
                    else:
                        nc.scalar.activation(et[:], pst[:], Exp, bias=ebias[:])
                    ets[si] = (et, g, t)
                # epilogue after this slot's exp
                if epi_pending is not None:
                    epilogue_batch(
                        epi_pending,
                        [pa_tiles.pop((epi_pending, qh)) for qh in range(NQH)],
                        [at_tiles.pop((epi_pending, qh)) for qh in range(NQH)],
                    )
                    epi_pending = None

    nc.compile()
    return nc


def _get_compiled(use_bias: bool):
    key = bool(use_bias)
    if key not in _compiled_cache:
        _compiled_cache[key] = _build(use_bias)
    return _compiled_cache[key]


def _prep(x, wq, bq, wk, bk, wv, bv, wo, bo):
    xf = np.ascontiguousarray(np.asarray(x, dtype=np.float32)).reshape(B, N, C)
    wq = np.asarray(wq, np.float32)
    bq = np.asarray(bq, np.float32)
    wk = np.asarray(wk, np.float32)
    bk = np.asarray(bk, np.float32)
    wv = np.asarray(wv, np.float32)
    bv = np.asarray(bv, np.float32)
    wo = np.asarray(wo, np.float32)
    bo = np.asarray(bo, np.float32)

    use_bias = not (np.all(bq == 0) and np.all(bk == 0) and np.all(bv == 0))

    scale = np.float32(1.0 / np.sqrt(np.float32(D)))
    wqs = wq * scale
    # lhsT tiles: [2 c-halves, 128, 64] with d replicated 2x along columns
    wqp = np.ascontiguousarray(np.tile(wqs.reshape(2, 128, D), (1, 1, PACK))).astype(BF16)
    wkp = np.ascontiguousarray(np.tile(wk.reshape(2, 128, D), (1, 1, PACK))).astype(BF16)
    if use_bias:
        # scores row-fold: r_k = x_k @ u, u = scale * (wk @ bq); exp(r) scales
        # key k's et column (bk and bq*bk terms drop out of softmax).
        u = (wk @ (bq * scale)).astype(np.float32)  # [C]
        wvx = np.concatenate([wv, u[:, None]], axis=1)  # [C, 257]
        wvp = np.ascontiguousarray(wvx.reshape(2, 128, 257)).astype(BF16)
    else:
        wvp = np.ascontiguousarray(wv.reshape(2, 128, 256)).astype(BF16)
    wop = np.ascontiguousarray(wo.reshape(2, 128, 256)).astype(BF16)

    in_maps = []
    for core in range(NCORES):
        b, h = divmod(core, 2)
        if h == 0:
            xo = xf[b]
        else:
            xo = np.concatenate([xf[b, NQ:], xf[b, :NQ]], 0)
        xT = np.ascontiguousarray(xo.T.reshape(2, 128, N)).astype(BF16)
        in_maps.append(
            {"xT16": xT, "wqp": wqp, "wkp": wkp, "wvp": wvp, "wop": wop}
        )
    # host residual fold: out += x + (bv @ wo + bo)
    resid_const = (bv.astype(np.float64) @ wo.astype(np.float64)).astype(np.float32) + bo
    return in_maps, use_bias, xf, resid_const


def _gather(results, xf, resid_const):
    out = np.empty((B, N, C), np.float32)
    for core in range(NCORES):
        b, h = divmod(core, 2)
        out[b, NQ * h : NQ * (h + 1)] = results[core]["out"]
    out += xf
    out += resid_const[None, None, :]
    return out.reshape(B, HH, WW, C)


def kernel(x, wq, bq, wk, bk, wv, bv, wo, bo):
    from concourse.bass_utils import run_bass_kernel_spmd

    in_maps, use_bias, xf, resid_const = _prep(x, wq, bq, wk, bk, wv, bv, wo, bo)
    nc = _get_compiled(use_bias)
    res = run_bass_kernel_spmd(nc, in_maps, core_ids=list(range(NCORES)))
    return _gather(res.results, xf, resid_const)


def _ensure_ntff_hook():
    """The agent image's antenv stub lacks axon_hooks; synthesize it so
    run_bass_kernel_spmd(trace=True) can NTFF-profile via libaxon_pjrt."""
    import types

    try:
        from antenv.axon_hooks import get_axon_ntff_profile_hook  # noqa: F401
        return
    except ImportError:
        pass
    import antenv
    from trn_agent_boot.trn_boot import _ntff_profile_via_ctypes

    mod = types.ModuleType("antenv.axon_hooks")
    state = {"h": _ntff_profile_via_ctypes("/opt/axon/libaxon_pjrt.so")}
    mod.get_axon_ntff_profile_hook = lambda: state["h"]
    mod.set_axon_ntff_profile_hook = lambda h: state.__setitem__("h", h)
    sys.modules["antenv.axon_hooks"] = mod
    antenv.axon_hooks = mod


def run_traced(inputs, **kw):
    """For test.py: run with NTFF profiling; returns (output, BassKernelResults)."""
    from concourse.bass_utils import run_bass_kernel_spmd

    _ensure_ntff_hook()

    in_maps, use_bias, xf, resid_const = _prep(**inputs)
    nc = _get_compiled(use_bias)
    res = run_bass_kernel_spmd(nc, in_maps, core_ids=list(range(NCORES)), trace=True, **kw)
    return _gather(res.results, xf, resid_const), res
